# revision 6
# baseline (speedup 1.0000x reference)
"""Trainium2 Bass kernel for nn_Convolution6D.

Math: the reference contracts per (l, l1, l2) CG-tensor triples against
ragged per-degree weights.  All of that folds (on host, tiny tensors) into a
single per-radial-index batched matmul:

    O[p][n, c] = X[p][n, f] @ W_eff[p][f, c] + bias[p][c]

with f = (l2, part_in, t, k, i) -> 4800 features (padded to 40 chunks x 128)
and  c = (part_out, l, lm, j)  -> 320 columns.

Device (per core, data-parallel over n, 1024 rows each):
  - gpsimd cast-DMA loads x tiles [128n, (t k) 5p 16i] f32->bf16 (natural,
    fully contiguous per row)
  - transpose 128x128 blocks via regular TensorE matmul with a bf16 identity
    as the moving operand (stationary = gathered x columns for one p)
  - PSUM->SBUF evacuation on Vector/Scalar engines (alternating)
  - accumulating matmuls into 5 per-p PSUM banks; bias folded in via a K=1
    ones-row matmul that opens each accumulation group
  - scatter copies to an [n, lm, p, j]-layout staging tile, then HWDGE DMA out
"""

import functools
import math

import numpy as np
import ml_dtypes

L = 4
T_ER, P_RAD, C_IN, C_OUT = 15, 5, 16, 16
N_PTS = 8192
NCORES = 8
ROWS = N_PTS // NCORES          # rows per core
NTILE = 128                     # n rows per tile
NCH = (2, 4, 6, 8)              # chunks of 128 features per (part, l2)
CHUNKS = [(part, l2, q) for l2 in range(L) for part in range(2)
          for q in range(NCH[l2])]          # 40 chunks -> F_pad = 5120
OFF_L = (0, 16, 48, 96)


# ---------------------------------------------------------------------------
# Clebsch-Gordan tables (identical math to the reference module)
# ---------------------------------------------------------------------------
def _cg(j1, m1, j2, m2, j3, m3):
    if m1 + m2 != m3:
        return 0.0
    if j3 < abs(j1 - j2) or j3 > j1 + j2:
        return 0.0
    f = math.factorial
    pre = math.sqrt((2 * j3 + 1) * f(j3 + j1 - j2) * f(j3 - j1 + j2)
                    * f(j1 + j2 - j3) / f(j1 + j2 + j3 + 1))
    pre *= math.sqrt(f(j3 + m3) * f(j3 - m3) * f(j1 - m1) * f(j1 + m1)
                     * f(j2 - m2) * f(j2 + m2))
    s = 0.0
    for k in range(max(0, j2 - j3 - m1, j1 - j3 + m2),
                   min(j1 + j2 - j3, j1 - m1, j2 + m2) + 1):
        s += (-1.0) ** k / (f(k) * f(j1 + j2 - j3 - k) * f(j1 - m1 - k)
                            * f(j2 + m2 - k) * f(j3 - j2 + m1 + k)
                            * f(j3 - j1 - m2 + k))
    return pre * s


@functools.lru_cache(maxsize=1)
def _precompute():
    tens, coefs = [], []
    for l in range(L):
        for l1 in range(L):
            for l2 in range(abs(l - l1), min(L, l + l1 + 1)):
                coefs.append(8.0 * np.pi ** 2 / (2 * l1 + 1)
                             * np.sqrt((2 * l + 1) * (2 * l1 + 1)
                                       / (4.0 * np.pi * (2 * l2 + 1)))
                             * _cg(l, 0, l1, 0, l2, 0))
                T1 = np.zeros((l + 1, l1 + 1, l2 + 1), np.float64)
                T2 = np.zeros_like(T1)
                T3 = np.zeros_like(T1)
                for k in range(l + 1):
                    for k1 in range(l1 + 1):
                        if k + k1 < l2 + 1:
                            T1[k, k1, k + k1] = _cg(l, k, l1, k1, l2, k + k1) * (-1) ** k1
                        if k1 > 0 and abs(k - k1) < l2 + 1:
                            if k - k1 >= 0:
                                T2[k, k1, k - k1] = _cg(l, k, l1, -k1, l2, k - k1) * (-1) ** l1
                            else:
                                T3[k, k1, k1 - k] = ((-1) ** (k1 - k) * (-1) ** (l1 + l2)
                                                     * _cg(l, k, l1, -k1, l2, k - k1))
                tens += [T1, T2, T3]
    return tens, [float(c) for c in coefs]


def _build_weff_bias(ws, bs):
    """Fold CG tensors + per-degree weights into Wt[128, 5, 40, 320] and
    bias[1, 5, 320] (both bf16). ws/bs indexed [2*l + (0 r | 1 i)]."""
    tens, coefs = _precompute()
    ws_m = [np.asarray(w, np.float64) for w in ws]
    bs_m = [np.asarray(b, np.float64) for b in bs]
    for l in range(L):
        mk = (np.arange(l + 1) > 0).astype(np.float64)
        ws_m[2 * l + 1] = ws_m[2 * l + 1] * mk[:, None, None, None, None]
        bs_m[2 * l + 1] = bs_m[2 * l + 1] * mk[None, :, None, None]

    A = {(part, l2): np.zeros((l2 + 1, T_ER, P_RAD, C_IN, 320), np.float64)
         for part in range(2) for l2 in range(L)}
    itr = 0
    for l in range(L):
        for l1 in range(L):
            wr = ws_m[2 * l1]
            wi = ws_m[2 * l1 + 1]
            for l2 in range(abs(l - l1), min(L, l + l1 + 1)):
                c = coefs[itr]
                T1, T2, T3 = tens[3 * itr], tens[3 * itr + 1], tens[3 * itr + 2]
                itr += 1
                for (part_in, Tm, w, part_out, sgn) in (
                    (0, T1 + T2 + T3, wr, 0, +c),
                    (1, T1 - T2 + T3, wi, 0, -c),
                    (0, T1 - T2 - T3, wi, 1, +c),
                    (1, T1 + T2 - T3, wr, 1, +c),
                ):
                    blk = np.einsum('lmk,mtpij->ktpilj', Tm, w) * sgn
                    col0 = part_out * 160 + OFF_L[l]
                    A[(part_in, l2)][:, :, :, :, col0: col0 + (l + 1) * 16] += \
                        blk.reshape(l2 + 1, T_ER, P_RAD, C_IN, (l + 1) * 16)

    Wt = np.zeros((128, P_RAD, len(CHUNKS), 320), np.float64)
    for ci, (part, l2, q) in enumerate(CHUNKS):
        a = A[(part, l2)].transpose(1, 0, 3, 2, 4)  # [t, k, i, p, c]
        a = a.reshape((l2 + 1) * T_ER, C_IN, P_RAD, 320)  # [(t k), i, p, c]
        g0, g1 = 8 * q, min(8 * q + 8, 15 * (l2 + 1))
        rows = a[g0:g1].reshape((g1 - g0) * C_IN, P_RAD, 320)
        Wt[: (g1 - g0) * C_IN, :, ci, :] = rows.transpose(0, 1, 2)

    bias = np.zeros((1, P_RAD, 320), np.float64)
    for l in range(L):
        for part_out in range(2):
            b = bs_m[2 * l + part_out]       # [1, lm, p, j]
            c0 = part_out * 160 + OFF_L[l]
            bias[0, :, c0: c0 + (l + 1) * 16] = \
                b[0].transpose(1, 0, 2).reshape(P_RAD, (l + 1) * 16)

    return (Wt.astype(ml_dtypes.bfloat16), bias.astype(ml_dtypes.bfloat16))


# ---------------------------------------------------------------------------
# Device program
# ---------------------------------------------------------------------------
@functools.lru_cache(maxsize=2)
def build_nc(rows=ROWS):
    import concourse.mybir as mybir
    import concourse.tile as tile
    from concourse import bacc
    from concourse.masks import make_identity

    ntiles = rows // NTILE
    nc = bacc.Bacc("TRN2", target_bir_lowering=False, num_devices=NCORES)
    xs_d = {}
    for l2 in range(L):
        for pi, pt in enumerate('ri'):
            # host pre-permuted to [n, p, (t k), i]
            xs_d[(pi, l2)] = nc.declare_dram_parameter(
                f'x{l2}{pt}', [rows, P_RAD, T_ER * (l2 + 1), C_IN],
                mybir.dt.float32, isOutput=False)
    w_d = nc.declare_dram_parameter('wpack', [128, P_RAD, len(CHUNKS), 320],
                                    mybir.dt.bfloat16, isOutput=False)
    b_d = nc.declare_dram_parameter('bpack', [1, P_RAD, 320],
                                    mybir.dt.bfloat16, isOutput=False)
    os_d = {}
    for l in range(L):
        for pi, pt in enumerate('ri'):
            os_d[(pi, l)] = nc.declare_dram_parameter(
                f'o{l}{pt}', [rows, l + 1, P_RAD, C_OUT],
                mybir.dt.float32, isOutput=True)

    with tile.TileContext(nc) as tc:
        with (
            tc.tile_pool(name="const", bufs=1) as constp,
            tc.tile_pool(name="nat", bufs=3) as natp,
            tc.tile_pool(name="xt", bufs=4) as xtp,
            tc.tile_pool(name="stg", bufs=2) as stgp,
            tc.tile_pool(name="pst", bufs=3, space="PSUM") as pstp,
            tc.tile_pool(name="pacc", bufs=1, space="PSUM") as paccp,
        ):
            w_sb = constp.tile([128, P_RAD, len(CHUNKS), 320],
                               mybir.dt.bfloat16, tag="w")
            nc.sync.dma_start(out=w_sb[:], in_=w_d[:])
            b_sb = constp.tile([1, P_RAD, 320], mybir.dt.bfloat16, tag="b")
            nc.sync.dma_start(out=b_sb[:], in_=b_d[:])
            ident = constp.tile([128, 128], mybir.dt.bfloat16, tag="id")
            make_identity(nc, ident)
            ones = constp.tile([1, 128], mybir.dt.bfloat16, tag="ones")
            nc.vector.memset(ones, 1.0)

            toggle = 0
            for it in range(ntiles):
                nsl = slice(it * NTILE, (it + 1) * NTILE)
                # per-p accumulators; bias opens each accumulation group
                accs = []
                for p in range(P_RAD):
                    acc = paccp.tile([128, 320], mybir.dt.float32, tag=f"acc{p}")
                    nc.tensor.matmul(acc[:], lhsT=ones[:], rhs=b_sb[:, p, :],
                                     start=True, stop=False)
                    accs.append(acc)

                for l2 in range(L):
                    G16 = 15 * (l2 + 1) * C_IN          # real cols per p
                    PREG = NCH[l2] * 128                # padded cols per p
                    for part in range(2):
                        natt = natp.tile([128, P_RAD, PREG],
                                         mybir.dt.bfloat16, tag="nat")
                        nc.vector.memset(natt[:, :, G16:], 0.0)
                        src = xs_d[(part, l2)][nsl].rearrange(
                            "n p g i -> n p (g i)")
                        nc.gpsimd.dma_start(out=natt[:, :, :G16], in_=src)

                        nchunk = NCH[l2]
                        ci0 = CHUNKS.index((part, l2, 0))
                        for p in range(P_RAD):
                            for q0 in range(0, nchunk, 4):
                                qn = min(4, nchunk - q0)
                                pst_t = pstp.tile([128, 512],
                                                  mybir.dt.float32, tag="pst")
                                xt_t = xtp.tile([128, 512],
                                                mybir.dt.bfloat16, tag="xt")
                                for j in range(qn):
                                    q = q0 + j
                                    nc.tensor.matmul(
                                        pst_t[:, j * 128:(j + 1) * 128],
                                        lhsT=natt[:, p, q * 128:(q + 1) * 128],
                                        rhs=ident[:],
                                        start=True, stop=True)
                                if toggle == 0:
                                    nc.vector.tensor_copy(
                                        xt_t[:, :qn * 128], pst_t[:, :qn * 128])
                                else:
                                    nc.scalar.copy(
                                        xt_t[:, :qn * 128], pst_t[:, :qn * 128])
                                toggle ^= 1
                                for j in range(qn):
                                    ci = ci0 + q0 + j
                                    last = (l2 == L - 1 and part == 1
                                            and q0 + j == nchunk - 1)
                                    nc.tensor.matmul(
                                        accs[p][:],
                                        lhsT=xt_t[:, j * 128:(j + 1) * 128],
                                        rhs=w_sb[:, p, ci, :],
                                        start=False, stop=last)

                # interleave accumulators into [n, lm(x part,l), p, j] staging
                stg_t = stgp.tile([128, 20, 80], mybir.dt.float32, tag="stg")
                for p in range(P_RAD):
                    for po in range(2):
                        for l in range(L):
                            c0 = po * 160 + OFF_L[l]
                            r0 = po * 10 + OFF_L[l] // 16
                            src = accs[p][:, c0:c0 + 16 * (l + 1)].rearrange(
                                "n (lm j) -> n lm j", j=16)
                            dst = stg_t[:, r0:r0 + (l + 1), p * 16:(p + 1) * 16]
                            if toggle == 0:
                                nc.vector.tensor_copy(dst, src)
                            else:
                                nc.scalar.copy(dst, src)
                            toggle ^= 1
                for po in range(2):
                    for l in range(L):
                        r0 = po * 10 + OFF_L[l] // 16
                        nc.sync.dma_start(out=os_d[(po, l)][nsl],
                                          in_=stg_t[:, r0:r0 + (l + 1), :])
    nc.compile()
    return nc


def _pack_inputs(inputs, rows=ROWS, ncores=NCORES):
    ws = [np.asarray(inputs[f'w{l}{pt}']) for l in range(L) for pt in 'ri']
    bs = [np.asarray(inputs[f'b{l}{pt}']) for l in range(L) for pt in 'ri']
    Wt, bias = _build_weff_bias(ws, bs)
    xperm = {}
    for l2 in range(L):
        for pt in 'ri':
            x = np.asarray(inputs[f'x{l2}{pt}'])
            n = x.shape[0]
            # [n, t, k, p, i] -> [n, p, (t k), i]
            xperm[f'x{l2}{pt}'] = np.ascontiguousarray(
                x.transpose(0, 3, 1, 2, 4).reshape(
                    n, P_RAD, T_ER * (l2 + 1), C_IN))
    in_maps = []
    for c in range(ncores):
        m = {'wpack': Wt, 'bpack': bias}
        for l2 in range(L):
            for pt in 'ri':
                m[f'x{l2}{pt}'] = xperm[f'x{l2}{pt}'][c * rows:(c + 1) * rows]
        in_maps.append(m)
    return in_maps


def kernel(**inputs):
    from concourse.bass_utils import run_bass_kernel_spmd

    nc = build_nc(ROWS)
    in_maps = _pack_inputs(inputs)
    res = run_bass_kernel_spmd(nc, in_maps, list(range(NCORES))).results
    outs = []
    for l in range(L):
        for pt in 'ri':
            outs.append(np.concatenate(
                [res[c][f'o{l}{pt}'] for c in range(NCORES)], axis=0))
    return tuple(outs)
